# revision 1
# baseline (speedup 1.0000x reference)
"""MoE routing kernel for Trainium2 (8 NeuronCores, SPMD data-parallel).

Computes, for x [4, 4096, 4096] f32, proto_k [64, 4096] f32, gate [64] f32:
    logits = relu(x @ proto_k.T / sqrt(4096) - gate)        # [B, S, 64]
    routing_weights, selected_experts = top_k(logits, k=8)  # [B, S, 8] each

Sharding: tokens (B*S = 16384) are split evenly across 8 cores (2048 each).
proto_k / gate are replicated. No collectives needed.

Numerics: the matmul runs as a 3-term fp16 hi/lo split (x = xh + xl,
proto = ph + pl, logits = xh@ph + xh@pl + xl@ph, dropping xl@pl ~ 2^-22).
The residuals are pre-scaled by 2^11 on the host so they stay in fp16's
normal range, accumulated in a second PSUM bank, and recombined as
hi + 2^-11 * lo on the DVE.  Validated: bit-noise-level agreement with the
fp32 reference (max logit perturbation ~4e-8, zero top-8 index flips),
while streaming the PE at fp16 rate (1 cycle/row, 3 passes) instead of
fp32's 4 cycles/row with serialized weight loads.

Per-core device program:
  - x shard is split/transposed on the host to xh/xl [4096, 2048] fp16 so
    every DMA is contiguous and the contraction dim rides SBUF partitions.
  - logits accumulate with experts on partitions: per 128-wide hidden chunk,
    3 matmuls into 2 PSUM banks ([64, 512] per 512-token group).
  - DVE recombines hi + 2^-11*lo; ScalarE applies relu(acc/64 - gate).
  - TensorE transposes [64, 128] tiles -> [128 tokens, 64 experts] PSUM.
  - DVE Max8/MaxIndex emit top-8 values (descending) + indices per token.
  - Outputs pack as [128, 16*8] tiles, unscrambled on the host.
"""

import numpy as np

HIDDEN = 4096
NUM_EXPERTS = 64
TOP_K = 8
N_CORES = 8
TOKENS = 4 * 4096
T_CORE = TOKENS // N_CORES          # 2048 tokens per core
N_CHUNK = HIDDEN // 128             # 32 contraction chunks
GROUPS_PER_PASS = 2                 # 512-token groups accumulated per pass
N_PASS = T_CORE // (512 * GROUPS_PER_PASS)
N_SUB = T_CORE // 128               # 16 output sub-tiles of 128 tokens
LO_SCALE = np.float32(2.0 ** 11)
LO_UNSCALE = 2.0 ** -11

_PROGRAM = None


def _split_multi_waits(nc):
    """walrus in this container rejects instructions carrying more sync waits
    than their ISA struct holds (setupSyncWait: 'Too many sync wait
    commands'); Drain takes one, S3_LW (matmul weight-load) ~two.  Normalize
    every instruction to a single wait by hoisting extras onto same-engine
    NOPs inserted immediately before the owner."""
    import bass_rust

    inserts = {}  # owner inst name -> list of wait-nop instructions
    for f in nc.m.functions:
        for bb in f.blocks:
            for inst in bb.instructions:
                si = inst.sync_info
                if si is None or len(si.on_wait) <= 1:
                    continue
                conds = list(si.on_wait)
                si.on_wait = conds[:1]
                eng = nc.engines[inst.engine]
                new_insts = []
                for w in conds[1:]:
                    nop = eng.nop(hint="split_wait")
                    nop.ins.sync_info = bass_rust.SyncInfo(
                        on_wait=[w], on_update=[]
                    )
                    new_insts.append(nop.ins)
                inserts[inst.name] = new_insts
    if not inserts:
        return
    # nop() appended the new instructions to whatever bb was current; strip
    # them from everywhere, then re-insert each right before its owner so
    # the engine observes every wait before executing the instruction.
    appended = {ni.name for nis in inserts.values() for ni in nis}
    for f in nc.m.functions:
        for bb in f.blocks:
            rebuilt = []
            changed = False
            for inst in bb.instructions:
                if inst.name in appended:
                    changed = True
                    continue
                if inst.name in inserts:
                    rebuilt.extend(inserts[inst.name])
                    changed = True
                rebuilt.append(inst)
            if changed:
                bb.instructions = rebuilt


def _build_program():
    import concourse.bass as bass
    import concourse.mybir as mybir
    import concourse.tile as tile

    f32 = mybir.dt.float32
    f16 = mybir.dt.float16
    u32 = mybir.dt.uint32
    E = NUM_EXPERTS

    nc = bass.Bass("TRN2", target_bir_lowering=False, debug=False)

    # xh and xl stacked: xhl[0] = hi, xhl[1] = lo (one DMA fetches both)
    xhl_d = nc.dram_tensor("xhl", [2, HIDDEN, T_CORE], f16, kind="ExternalInput")
    # proto hi|lo packed along expert columns: [:, 0:64] = ph, [:, 64:128] = pl
    phpl_d = nc.dram_tensor("phpl", [HIDDEN, 2 * E], f16, kind="ExternalInput")
    gate_neg = nc.dram_tensor("gate_neg", [E, 1], f32, kind="ExternalInput")
    w_out = nc.dram_tensor("w_out", [128, N_SUB * TOP_K], f32, kind="ExternalOutput")
    i_out = nc.dram_tensor("i_out", [128, N_SUB * TOP_K], u32, kind="ExternalOutput")

    ident_dram = nc.inline_tensor(np.eye(E, dtype=np.float32), name="ident64")

    with tile.TileContext(nc) as tc:
        with (
            tc.tile_pool(name="const", bufs=1) as const_pool,
            tc.tile_pool(name="xa", bufs=12) as x_pool,
            tc.tile_pool(name="acc", bufs=7, space="PSUM") as acc_pool,
            tc.tile_pool(name="tp", bufs=1, space="PSUM") as tp_pool,
            tc.tile_pool(name="lg", bufs=3) as lg_pool,
            tc.tile_pool(name="tk", bufs=3) as tk_pool,
            tc.tile_pool(name="outp", bufs=1) as out_pool,
        ):
            # --- constants ---
            # proto chunks land as [128, c, E]; per-chunk DMAs are contiguous
            # 32 KB and let the first matmuls start early.
            # weights ride the (otherwise idle) gpsimd SWDGE ring so neither
            # the x stream (sync ring) nor the epilogue traffic (scalar
            # ring) queues behind their 32 triggers.
            phpl_sb = const_pool.tile([128, N_CHUNK * 2 * E], f16)
            for c in range(N_CHUNK):
                nc.gpsimd.dma_start(
                    phpl_sb[:, c * 2 * E:(c + 1) * 2 * E],
                    phpl_d[c * 128:(c + 1) * 128, :],
                )
            gate_sb = const_pool.tile([E, 1], f32)
            nc.scalar.dma_start(gate_sb[:], gate_neg[:])
            ident_sb = const_pool.tile([E, E], f32)
            nc.scalar.dma_start(ident_sb[:], ident_dram[:])

            vals_sb = out_pool.tile([128, N_SUB * TOP_K], f32)
            idx_sb = out_pool.tile([128, N_SUB * TOP_K], u32)

            for p in range(N_PASS):
                tpp = GROUPS_PER_PASS * 512
                t0 = p * tpp
                # a = xh @ [ph|pl]: rows 0:64 main term, 64:128 lo (2^11)
                # b = xl @ [ph|pl]: rows 0:64 lo (2^11), 64:128 llo (2^22)
                a_accs = [
                    acc_pool.tile([128, 512], f32, name=f"a_p{p}g{g}", tag="acc")
                    for g in range(GROUPS_PER_PASS)
                ]
                b_accs = [
                    acc_pool.tile([128, 512], f32, name=f"b_p{p}g{g}", tag="acc")
                    for g in range(GROUPS_PER_PASS)
                ]
                for c in range(N_CHUNK):
                    # one HWDGE DMA per chunk fetches hi and lo halves;
                    # alternate between the two HWDGE rings (SP / ACT) so
                    # trigger issue is never the bottleneck
                    x_t = x_pool.tile([128, 2, tpp], f16, name="x_t", tag="xt")
                    src = (xhl_d[:, c * 128:(c + 1) * 128, t0:t0 + tpp]
                           .rearrange("s p t -> p s t"))
                    if p == 0 and c == 0:
                        # split the very first chunk by stream and group
                        # across both rings: the first matmul then waits on
                        # a 128 KB transfer instead of 512 KB
                        nc.sync.dma_start(x_t[:, 0, 0:512], src[:, 0, 0:512])
                        nc.scalar.dma_start(x_t[:, 1, 0:512], src[:, 1, 0:512])
                        nc.sync.dma_start(x_t[:, 0, 512:tpp], src[:, 0, 512:tpp])
                        nc.scalar.dma_start(x_t[:, 1, 512:tpp], src[:, 1, 512:tpp])
                    else:
                        ring = nc.sync if c % 2 == 0 else nc.scalar
                        ring.dma_start(x_t[:], src)
                    first, last = (c == 0), (c == N_CHUNK - 1)
                    pc = slice(c * 2 * E, (c + 1) * 2 * E)
                    # on the final chunk of the final pass, close the groups
                    # in reverse so the tail-critical epilogue starts while
                    # the other group's last matmuls still run
                    grange = (reversed(range(GROUPS_PER_PASS))
                              if (last and p == N_PASS - 1)
                              else range(GROUPS_PER_PASS))
                    for g in grange:
                        ts = slice(g * 512, (g + 1) * 512)
                        nc.tensor.matmul(
                            a_accs[g][:], phpl_sb[:, pc], x_t[:, 0, ts],
                            start=first, stop=last,
                        )
                        nc.tensor.matmul(
                            b_accs[g][:], phpl_sb[:, pc], x_t[:, 1, ts],
                            start=first, stop=last,
                        )
                erange = (list(reversed(range(GROUPS_PER_PASS)))
                          if p == N_PASS - 1 else list(range(GROUPS_PER_PASS)))
                for g in erange:
                    # comb = a[0:64] + 2^-11*(a[64:128] + b[0:64] + 2^-11*b[64:128])
                    # DVE reads at most one PSUM input per op, so `a` is
                    # staged through SBUF (which also releases its PSUM bank
                    # for the next pass early).  The reads of the [64:128]
                    # halves into 0:64-partition outputs are cross-partition
                    # APs — verified exact on hardware.
                    a_sb = lg_pool.tile([128, 512], f32, name="a_sb")
                    nc.vector.tensor_copy(a_sb[:], a_accs[g][:])
                    u = lg_pool.tile([E, 512], f32, name="u")
                    nc.vector.scalar_tensor_tensor(
                        u[:], b_accs[g][0:E, :], 1.0, a_sb[E:2 * E, :],
                        bass.mybir.AluOpType.mult, bass.mybir.AluOpType.add,
                    )
                    v = lg_pool.tile([E, 512], f32, name="v")
                    nc.vector.scalar_tensor_tensor(
                        v[:], b_accs[g][E:2 * E, :], LO_UNSCALE, u[:],
                        bass.mybir.AluOpType.mult, bass.mybir.AluOpType.add,
                    )
                    comb = lg_pool.tile([E, 512], f32, name="comb")
                    nc.vector.scalar_tensor_tensor(
                        comb[:], v[:], LO_UNSCALE, a_sb[0:E, :],
                        bass.mybir.AluOpType.mult, bass.mybir.AluOpType.add,
                    )
                    # relu(acc/64 - gate)  (ScalarE, SBUF -> SBUF)
                    logits = lg_pool.tile([E, 512], f32, name="logits")
                    nc.scalar.activation(
                        logits[:], comb[:],
                        bass.mybir.ActivationFunctionType.Relu,
                        bias=gate_sb[:], scale=1.0 / 64.0,
                    )
                    # transpose to [128 tokens, 64 experts] x 4 sub-tiles
                    tk_psum = tp_pool.tile([128, 4 * E], f32, name="tk_psum")
                    for j in range(4):
                        nc.tensor.transpose(
                            tk_psum[:, j * E:(j + 1) * E],
                            logits[:, j * 128:(j + 1) * 128],
                            ident_sb[:],
                        )
                    tk_sb = tk_pool.tile([128, 4 * E], f32, name="tk_sb")
                    nc.vector.tensor_copy(tk_sb[:], tk_psum[:])
                    gg = p * GROUPS_PER_PASS + g
                    for j in range(4):
                        s = gg * 4 + j
                        nc.vector.max(
                            vals_sb[:, s * TOP_K:(s + 1) * TOP_K],
                            tk_sb[:, j * E:(j + 1) * E],
                        )
                        nc.vector.max_index(
                            idx_sb[:, s * TOP_K:(s + 1) * TOP_K],
                            vals_sb[:, s * TOP_K:(s + 1) * TOP_K],
                            tk_sb[:, j * E:(j + 1) * E],
                        )
                # flush this pass's outputs so only the last pass's epilogue
                # sits in the kernel tail
                os_ = slice(p * GROUPS_PER_PASS * 4 * TOP_K,
                            (p + 1) * GROUPS_PER_PASS * 4 * TOP_K)
                nc.scalar.dma_start(w_out[:, os_], vals_sb[:, os_])
                nc.scalar.dma_start(i_out[:, os_], idx_sb[:, os_])

    _split_multi_waits(nc)
    return nc


def _get_program():
    global _PROGRAM
    if _PROGRAM is None:
        _PROGRAM = _build_program()
    return _PROGRAM


def _make_in_maps(x, proto_k, gate):
    xf = np.ascontiguousarray(x, dtype=np.float32).reshape(TOKENS, HIDDEN)
    proto = np.asarray(proto_k, dtype=np.float32)
    ph = proto.astype(np.float16)
    pl = ((proto - ph.astype(np.float32)) * LO_SCALE).astype(np.float16)
    phpl = np.concatenate([ph.T, pl.T], axis=1)           # [4096, 128] f16
    gate_neg = np.ascontiguousarray(
        -np.asarray(gate, dtype=np.float32).reshape(NUM_EXPERTS, 1)
    )
    in_maps = []
    for c in range(N_CORES):
        shard_t = xf[c * T_CORE:(c + 1) * T_CORE].T       # [4096, 2048] view
        xhl = np.empty((2, HIDDEN, T_CORE), np.float16)
        xhl[0] = shard_t
        xhl[1] = (shard_t - xhl[0].astype(np.float32)) * LO_SCALE
        in_maps.append(
            {"xhl": xhl, "phpl": phpl, "gate_neg": gate_neg}
        )
    return in_maps


def _gather(results):
    w = np.empty((TOKENS, TOP_K), np.float32)
    idx = np.empty((TOKENS, TOP_K), np.int32)
    for c in range(N_CORES):
        wo = results[c]["w_out"]                          # [128, 16*8]
        io = results[c]["i_out"].view(np.int32)
        w[c * T_CORE:(c + 1) * T_CORE] = (
            wo.reshape(128, N_SUB, TOP_K).transpose(1, 0, 2).reshape(T_CORE, TOP_K)
        )
        idx[c * T_CORE:(c + 1) * T_CORE] = (
            io.reshape(128, N_SUB, TOP_K).transpose(1, 0, 2).reshape(T_CORE, TOP_K)
        )
    return w.reshape(4, 4096, TOP_K), idx.reshape(4, 4096, TOP_K)


def run_sharded(in_maps, trace=False, trace_cores=None):
    from concourse.bass_utils import run_bass_kernel_spmd

    nc = _get_program()
    return run_bass_kernel_spmd(
        nc,
        in_maps,
        core_ids=list(range(N_CORES)),
        trace=trace,
        trace_cores=trace_cores,
    )


def kernel(x, proto_k, gate):
    in_maps = _make_in_maps(x, proto_k, gate)
    res = run_sharded(in_maps, trace=False)
    return _gather(res.results)



# revision 3
# speedup vs baseline: 1.0054x; 1.0054x over previous
"""MoE routing kernel for Trainium2 (8 NeuronCores, SPMD data-parallel).

Computes, for x [4, 4096, 4096] f32, proto_k [64, 4096] f32, gate [64] f32:
    logits = relu(x @ proto_k.T / sqrt(4096) - gate)        # [B, S, 64]
    routing_weights, selected_experts = top_k(logits, k=8)  # [B, S, 8] each

Sharding: tokens (B*S = 16384) split evenly across 8 cores (2048 each);
proto_k / gate replicated. No collectives.

Numerics: fp16 hi/lo split (x = xh + xl, proto = ph + pl, residuals
pre-scaled by 2^11 to stay fp16-normal).  Both streams use the full
[ph | pl] 128-column weight tile (a = xh@[ph|pl], b = xl@[ph|pl]); the
epilogue reads 3 terms (hi, xh@pl, xl@ph) and drops the 2^-22 xl@pl rows
(bounded 4e-9; validated zero top-8 flips vs the fp32 reference).
Full-width weight tiles matter for speed: 64-col LDWEIGHTS disables the
PE's background-weight-buffer pull-ahead and serializes LDW+MM (~390ns/MM
instead of ~260ns).

Schedule, built for DMA-roofline streaming (~358 GB/s/core):
  - tokens split into 3 PAIRS of groups: (512,512), (256,256), (256,256).
    The x stream is delivered pair-major (all 32 hidden-chunks of pair 0,
    then pair 1, ...), so pair accumulations close staggered and every
    epilogue except the last pair's hides under the next pair's stream.
    N>=256 matmuls keep the PE ahead of the stream even at the cold
    (1.2GHz) clock; tile sizes taper at the stream's ends.
  - 24 warm-up matmuls on the weight tile bridge the preamble so the
    HAM clock gate unthrottles before the first real matmul (without
    them the kernel is bistable: a ramp idle >1.4us throttles the PE and
    a ~12us-slower cold equilibrium can persist).
  - per chunk the 4 matmuls rotate over 4 PSUM banks (aA, bA, aB, bB) so
    consecutive matmuls never revisit a bank within <4 issues (PSUM
    drain turnaround would stall the PE).
  - x is host-packed into the exact delivery layout: each DMA tile is
    [128, 2 chunks x (hi,lo) x pair-tokens] contiguous per partition in
    HBM (8KB -> 4KB descriptors), two HWDGE rings alternating; proto
    weights go as 4 quarters interleaved with the first x tiles so
    neither ring's startup gates the first matmul.
  - epilogue per group in <=256-token slices: ScalarE copy(hi)/64 ->
    DVE STT (+2^-11/64 * xh@pl) -> DVE STT (+2^-11/64 * xl@ph) ->
    ScalarE relu(. - gate) -> PE transpose to [128 tok, 64 expert] ->
    DVE Max8/MaxIndex -> per-pair output flush.
"""

import numpy as np

HIDDEN = 4096
NUM_EXPERTS = 64
TOP_K = 8
N_CORES = 8
TOKENS = 4 * 4096
T_CORE = TOKENS // N_CORES          # 2048 tokens per core
N_CHUNK = HIDDEN // 128             # 32 contraction chunks
# (token offset, pair width, per-tile chunk counts).  Steady-state tiles are
# 8KB/partition = 1MB so DMA sem-lane pipelining (8 lanes, ~2us completion
# receipt each) keeps ~4x margin over the HBM rate; the first and last
# tiles taper down so the first matmul starts early and almost no matmul
# work remains after the final tile's completion sem fires.
PAIRS = (
    (0, 1024, (1, 1, 1, 1) + (2,) * 14),
    (1024, 512, (4,) * 8),
    (1536, 512, (4,) * 7 + (2, 1, 1)),
)
HOIST = 2                           # next-pair tiles prefetched before boundary
N_SUB = T_CORE // 128               # 16 output sub-tiles of 128 tokens
LO_SCALE = np.float32(2.0 ** 11)
LO_UNSCALE = 2.0 ** -11

_PROGRAM = None


def _split_multi_waits(nc):
    """walrus in this container rejects instructions carrying more sync waits
    than their ISA struct holds; normalize to a single wait by hoisting
    extras onto same-engine NOPs inserted immediately before the owner."""
    import bass_rust

    inserts = {}
    for f in nc.m.functions:
        for bb in f.blocks:
            for inst in bb.instructions:
                si = inst.sync_info
                if si is None or len(si.on_wait) <= 1:
                    continue
                conds = list(si.on_wait)
                si.on_wait = conds[:1]
                eng = nc.engines[inst.engine]
                new_insts = []
                for w in conds[1:]:
                    nop = eng.nop(hint="split_wait")
                    nop.ins.sync_info = bass_rust.SyncInfo(
                        on_wait=[w], on_update=[]
                    )
                    new_insts.append(nop.ins)
                inserts[inst.name] = new_insts
    if not inserts:
        return
    appended = {ni.name for nis in inserts.values() for ni in nis}
    for f in nc.m.functions:
        for bb in f.blocks:
            rebuilt = []
            changed = False
            for inst in bb.instructions:
                if inst.name in appended:
                    changed = True
                    continue
                if inst.name in inserts:
                    rebuilt.extend(inserts[inst.name])
                    changed = True
                rebuilt.append(inst)
            if changed:
                bb.instructions = rebuilt


def _build_program():
    import concourse.bass as bass
    import concourse.mybir as mybir
    import concourse.tile as tile

    f32 = mybir.dt.float32
    f16 = mybir.dt.float16
    u32 = mybir.dt.uint32
    E = NUM_EXPERTS

    nc = bass.Bass("TRN2", target_bir_lowering=False, debug=False)

    total_cols = N_CHUNK * 2 * T_CORE                     # 131072 f16 / part
    xdev = nc.dram_tensor("xdev", [128, total_cols], f16, kind="ExternalInput")
    phpl_d = nc.dram_tensor("phpl", [128, N_CHUNK * 2 * E], f16,
                            kind="ExternalInput")
    gate_neg = nc.dram_tensor("gate_neg", [E, 1], f32, kind="ExternalInput")
    w_out = nc.dram_tensor("w_out", [128, N_SUB * TOP_K], f32, kind="ExternalOutput")
    i_out = nc.dram_tensor("i_out", [128, N_SUB * TOP_K], u32, kind="ExternalOutput")

    ident_dram = nc.inline_tensor(np.eye(E, dtype=np.float32), name="ident64")

    with tile.TileContext(nc) as tc:
        with (
            tc.tile_pool(name="const", bufs=1) as const_pool,
            tc.tile_pool(name="xa", bufs=14) as x_pool,
            tc.tile_pool(name="acc", bufs=4, space="PSUM") as acc_pool,
            tc.tile_pool(name="bacc", bufs=3, space="PSUM") as b_pool,
            tc.tile_pool(name="tp", bufs=1, space="PSUM") as tp_pool,
            tc.tile_pool(name="lg", bufs=4) as lg_pool,
            tc.tile_pool(name="tk", bufs=4) as tk_pool,
            tc.tile_pool(name="outp", bufs=1) as out_pool,
        ):
            # --- constants.  Proto weights go as quarters interleaved with
            # the first x tiles on both rings, so weight delivery stays ~8
            # chunks ahead of the x stream without gating the first matmul.
            phpl_sb = const_pool.tile([128, N_CHUNK * 2 * E], f16)
            WQ = N_CHUNK // 4 * 2 * E                     # weight quarter cols
            nc.sync.dma_start(phpl_sb[:, 0:WQ], phpl_d[:, 0:WQ])
            gate_sb = const_pool.tile([E, 1], f32)
            nc.scalar.dma_start(gate_sb[:], gate_neg[:])
            ident_sb = const_pool.tile([E, E], f32)
            nc.scalar.dma_start(ident_sb[:], ident_dram[:])

            vals_sb = out_pool.tile([128, N_SUB * TOP_K], f32)
            idx_sb = out_pool.tile([128, N_SUB * TOP_K], u32)

            # PE warm-up: the HAM clock gate boots at 1.2GHz and needs
            # ~3.4us of sustained matmul activity to unthrottle.  Without
            # this, whether the kernel lands in the warm-PE equilibrium or a
            # ~12us-slower HAM-cold one depends on ramp timing luck.  These
            # matmuls depend only on the first weight quarter (lands ~10us,
            # while the first x tile is still in flight) and write a scratch
            # PSUM tile nobody reads.
            warm = tp_pool.tile([128, 4 * E], f32, name="warm", tag="tkp")
            for w in range(24):
                nc.tensor.matmul(
                    warm[:, :], phpl_sb[:, 0:128], phpl_sb[:, 0:4 * E],
                    start=(w == 0), stop=(w == 23),
                )

            # x tile column offsets in xdev (pair-major layout)
            tile_off = {}
            o = 0
            for p, (toff, pw, cpts) in enumerate(PAIRS):
                for j, cpt in enumerate(cpts):
                    tile_off[(p, j)] = o
                    o += cpt * 2 * pw

            prefetched = {}

            def emit_xdma(p, j):
                # all x triggers ride the sync ring: the scalar ENGINE runs
                # the epilogue ACTs (a trigger queued behind them would stall
                # the stream at pair boundaries), and the scalar ring's first
                # packets start ~3.4us late, which perturbs the ramp enough
                # to trip the HAM clock gate
                pw, cpt = PAIRS[p][1], PAIRS[p][2][j]
                tile_cols = cpt * 2 * pw
                off = tile_off[(p, j)]
                x_t = x_pool.tile([128, 4096], f16, name="x_t", tag="xt")
                nc.sync.dma_start(x_t[:, 0:tile_cols],
                                  xdev[:, off:off + tile_cols])
                return x_t

            for p, (toff, pw, cpts) in enumerate(PAIRS):
                W = pw // 2
                tpp = len(cpts)
                accs = [
                    acc_pool.tile([128, W], f32, name=f"acc_p{p}{h}", tag="acc")
                    for h in range(2)
                ]
                baccs = [
                    b_pool.tile([128, W], f32, name=f"bacc_p{p}{h}", tag="bacc")
                    for h in range(2)
                ]
                c0 = 0
                for j in range(tpp):
                    cpt = cpts[j]
                    # hoist the next pair's first tiles ahead of this pair's
                    # last tiles so the PE sees no delivery gap at the
                    # boundary (a >1us PE idle trips the HAM clock gate)
                    if p + 1 < len(PAIRS) and j >= tpp - HOIST:
                        hj = j - (tpp - HOIST)
                        prefetched[(p + 1, hj)] = emit_xdma(p + 1, hj)
                    if (p, j) in prefetched:
                        x_t = prefetched.pop((p, j))
                    else:
                        x_t = emit_xdma(p, j)
                    if p == 0 and j in (0, 1, 2):
                        q = j + 1
                        nc.sync.dma_start(
                            phpl_sb[:, q * WQ:(q + 1) * WQ],
                            phpl_d[:, q * WQ:(q + 1) * WQ])
                    for k in range(cpt):
                        c = c0 + k
                        pc = slice(c * 2 * E, (c + 1) * 2 * E)
                        first, last = (c == 0), (c == N_CHUNK - 1)
                        for h in range(2):
                            rh = x_t[:, (2 * k) * pw + h * W:
                                     (2 * k) * pw + (h + 1) * W]
                            rl = x_t[:, (2 * k + 1) * pw + h * W:
                                     (2 * k + 1) * pw + (h + 1) * W]
                            nc.tensor.matmul(
                                accs[h][:, :], phpl_sb[:, pc], rh,
                                start=first, stop=last,
                            )
                            nc.tensor.matmul(
                                baccs[h][:, :], phpl_sb[:, pc], rl,
                                start=first, stop=last,
                            )
                    c0 += cpt
                # epilogue per group in <=256-col slices (128-aligned)
                for h in range(2):
                    acc, bacc = accs[h], baccs[h]
                    nsub = W // 128
                    tkp = tp_pool.tile([128, nsub * E], f32,
                                       name=f"tk_psum_p{p}{h}", tag="tkp")
                    s0 = (toff + h * W) // 128
                    for e0 in range(0, W, 256):
                        ew = min(256, W - e0)
                        hs = slice(e0, e0 + ew)
                        hi_sb = lg_pool.tile([E, 256], f32, name="hi_sb")
                        nc.scalar.activation(
                            hi_sb[0:E, 0:ew], acc[0:E, hs],
                            bass.mybir.ActivationFunctionType.Copy,
                            scale=1.0 / 64.0,
                        )
                        u_sb = lg_pool.tile([E, 256], f32, name="u_sb")
                        nc.vector.scalar_tensor_tensor(
                            u_sb[0:E, 0:ew], acc[E:2 * E, hs],
                            LO_UNSCALE / 64.0, hi_sb[0:E, 0:ew],
                            bass.mybir.AluOpType.mult, bass.mybir.AluOpType.add,
                        )
                        comb = lg_pool.tile([E, 256], f32, name="comb")
                        nc.vector.scalar_tensor_tensor(
                            comb[0:E, 0:ew], bacc[0:E, hs],
                            LO_UNSCALE / 64.0, u_sb[0:E, 0:ew],
                            bass.mybir.AluOpType.mult, bass.mybir.AluOpType.add,
                        )
                        logits = lg_pool.tile([E, 256], f32, name="logits")
                        nc.scalar.activation(
                            logits[0:E, 0:ew], comb[0:E, 0:ew],
                            bass.mybir.ActivationFunctionType.Relu,
                            bias=gate_sb[:], scale=1.0,
                        )
                        for q in range(ew // 128):
                            sl = (e0 + q * 128) // 128
                            nc.tensor.transpose(
                                tkp[:, sl * E:(sl + 1) * E],
                                logits[:, q * 128:(q + 1) * 128],
                                ident_sb[:],
                            )
                        # copy transposes out of PSUM promptly (the PE's next
                        # transpose into tkp must not wait on DVE max ops —
                        # it would stall the in-order PE queue)
                        tk_sb = tk_pool.tile([128, 2 * E], f32, name="tk_sb")
                        nc.vector.tensor_copy(
                            tk_sb[:, 0:(ew // 128) * E],
                            tkp[:, (e0 // 128) * E:((e0 + ew) // 128) * E])
                        for q in range(ew // 128):
                            s = s0 + (e0 + q * 128) // 128
                            nc.vector.max(
                                vals_sb[:, s * TOP_K:(s + 1) * TOP_K],
                                tk_sb[:, q * E:(q + 1) * E],
                            )
                            nc.vector.max_index(
                                idx_sb[:, s * TOP_K:(s + 1) * TOP_K],
                                vals_sb[:, s * TOP_K:(s + 1) * TOP_K],
                                tk_sb[:, q * E:(q + 1) * E],
                            )
                    if p < len(PAIRS) - 1:
                        # mid-stream flushes ride the (otherwise idle) gpsimd
                        # SWDGE ring so they queue behind neither the x
                        # stream nor the ACTs
                        os_ = slice(s0 * TOP_K, (s0 + W // 128) * TOP_K)
                        nc.gpsimd.dma_start(w_out[:, os_], vals_sb[:, os_])
                        nc.gpsimd.dma_start(i_out[:, os_], idx_sb[:, os_])
                if p == len(PAIRS) - 1:
                    # final flush: one trigger per tensor on the scalar ring
                    # (HWDGE descgen is ~300ns vs gpsimd's 640, and scalar is
                    # idle by now) — this sits on the kernel's critical tail
                    os_ = slice(toff // 128 * TOP_K, (toff + pw) // 128 * TOP_K)
                    nc.scalar.dma_start(w_out[:, os_], vals_sb[:, os_])
                    nc.scalar.dma_start(i_out[:, os_], idx_sb[:, os_])

    _split_multi_waits(nc)
    return nc


def _get_program():
    global _PROGRAM
    if _PROGRAM is None:
        _PROGRAM = _build_program()
    return _PROGRAM


def _make_in_maps(x, proto_k, gate):
    xf = np.ascontiguousarray(x, dtype=np.float32).reshape(TOKENS, HIDDEN)
    proto = np.asarray(proto_k, dtype=np.float32)
    ph = proto.astype(np.float16)
    pl = ((proto - ph.astype(np.float32)) * LO_SCALE).astype(np.float16)
    # phpl[p, c*128+m]: W = [ph; pl] rows = 128 packed expert cols
    Wm = np.concatenate([ph, pl], axis=0)                 # [128, 4096]
    phpl = np.ascontiguousarray(
        Wm.T.reshape(N_CHUNK, 128, 2 * NUM_EXPERTS).transpose(1, 0, 2)
        .reshape(128, N_CHUNK * 2 * NUM_EXPERTS)
    )
    gate_neg = np.ascontiguousarray(
        -np.asarray(gate, dtype=np.float32).reshape(NUM_EXPERTS, 1)
    )
    in_maps = []
    for cid in range(N_CORES):
        xs = xf[cid * T_CORE:(cid + 1) * T_CORE]          # [2048, 4096]
        xh = xs.astype(np.float16)
        xl = ((xs - xh.astype(np.float32)) * LO_SCALE).astype(np.float16)
        # A[c, p, s, t] = (xh if s==0 else xl)[t, c*128+p]
        A = np.empty((N_CHUNK, 128, 2, T_CORE), np.float16)
        A[:, :, 0, :] = xh.T.reshape(N_CHUNK, 128, T_CORE)
        A[:, :, 1, :] = xl.T.reshape(N_CHUNK, 128, T_CORE)
        # delivery: pair-major, tiles of cpts[j] chunks: cols [p][j][k][s][t]
        parts = []
        for toff, pw, cpts in PAIRS:
            c0 = 0
            for cpt in cpts:
                blk = A[c0:c0 + cpt, :, :, toff:toff + pw]
                parts.append(blk.transpose(1, 0, 2, 3).reshape(128, -1))
                c0 += cpt
        xd = np.ascontiguousarray(np.concatenate(parts, axis=1))
        in_maps.append({"xdev": xd, "phpl": phpl, "gate_neg": gate_neg})
    return in_maps


def _gather(results):
    w = np.empty((TOKENS, TOP_K), np.float32)
    idx = np.empty((TOKENS, TOP_K), np.int32)
    for c in range(N_CORES):
        wo = results[c]["w_out"]                          # [128, 16*8]
        io = results[c]["i_out"].view(np.int32)
        w[c * T_CORE:(c + 1) * T_CORE] = (
            wo.reshape(128, N_SUB, TOP_K).transpose(1, 0, 2).reshape(T_CORE, TOP_K)
        )
        idx[c * T_CORE:(c + 1) * T_CORE] = (
            io.reshape(128, N_SUB, TOP_K).transpose(1, 0, 2).reshape(T_CORE, TOP_K)
        )
    return w.reshape(4, 4096, TOP_K), idx.reshape(4, 4096, TOP_K)


def run_sharded(in_maps, trace=False, trace_cores=None):
    from concourse.bass_utils import run_bass_kernel_spmd

    nc = _get_program()
    return run_bass_kernel_spmd(
        nc,
        in_maps,
        core_ids=list(range(N_CORES)),
        trace=trace,
        trace_cores=trace_cores,
    )


def kernel(x, proto_k, gate):
    in_maps = _make_in_maps(x, proto_k, gate)
    res = run_sharded(in_maps, trace=False)
    return _gather(res.results)


# revision 5
# speedup vs baseline: 1.0109x; 1.0055x over previous
"""MoE routing kernel for Trainium2 (8 NeuronCores, SPMD data-parallel).

Computes, for x [4, 4096, 4096] f32, proto_k [64, 4096] f32, gate [64] f32:
    logits = relu(x @ proto_k.T / sqrt(4096) - gate)        # [B, S, 64]
    routing_weights, selected_experts = top_k(logits, k=8)  # [B, S, 8] each

Sharding: tokens (B*S = 16384) split evenly across 8 cores (2048 each);
proto_k / gate replicated. No collectives.

Numerics: fp16 hi/lo split (x = xh + xl, proto = ph + pl, residuals
pre-scaled by 2^11 to stay fp16-normal).  Both streams use the full
[ph | pl] 128-column weight tile (a = xh@[ph|pl], b = xl@[ph|pl]); the
epilogue reads 3 terms (hi, xh@pl, xl@ph) and drops the 2^-22 xl@pl rows
(bounded 4e-9; validated zero top-8 flips vs the fp32 reference).
Full-width weight tiles matter for speed: 64-col LDWEIGHTS disables the
PE's background-weight-buffer pull-ahead and serializes LDW+MM (~390ns/MM
instead of ~260ns).

Schedule, built for DMA-roofline streaming (~358 GB/s/core):
  - tokens split into 3 PAIRS of groups: (512,512), (256,256), (256,256).
    The x stream is delivered pair-major (all 32 hidden-chunks of pair 0,
    then pair 1, ...), so pair accumulations close staggered and every
    epilogue except the last pair's hides under the next pair's stream.
    N>=256 matmuls keep the PE ahead of the stream even at the cold
    (1.2GHz) clock; tile sizes taper at the stream's ends.
  - 24 warm-up matmuls on the weight tile bridge the preamble so the
    HAM clock gate unthrottles before the first real matmul (without
    them the kernel is bistable: a ramp idle >1.4us throttles the PE and
    a ~12us-slower cold equilibrium can persist).
  - per chunk the 4 matmuls rotate over 4 PSUM banks (aA, bA, aB, bB) so
    consecutive matmuls never revisit a bank within <4 issues (PSUM
    drain turnaround would stall the PE).
  - x is host-packed into the exact delivery layout: each DMA tile is
    [128, 2 chunks x (hi,lo) x pair-tokens] contiguous per partition in
    HBM (8KB -> 4KB descriptors), two HWDGE rings alternating; proto
    weights go as 4 quarters interleaved with the first x tiles so
    neither ring's startup gates the first matmul.
  - epilogue per group in <=256-token slices: ScalarE copy(hi)/64 ->
    DVE STT (+2^-11/64 * xh@pl) -> DVE STT (+2^-11/64 * xl@ph) ->
    ScalarE relu(. - gate) -> PE transpose to [128 tok, 64 expert] ->
    DVE Max8/MaxIndex -> per-pair output flush.
"""

import numpy as np

HIDDEN = 4096
NUM_EXPERTS = 64
TOP_K = 8
N_CORES = 8
TOKENS = 4 * 4096
T_CORE = TOKENS // N_CORES          # 2048 tokens per core
N_CHUNK = HIDDEN // 128             # 32 contraction chunks
# (token offset, pair width, per-tile chunk counts).  Steady-state tiles are
# 8KB/partition = 1MB so DMA sem-lane pipelining (8 lanes, ~2us completion
# receipt each) keeps ~4x margin over the HBM rate; the first and last
# tiles taper down so the first matmul starts early and almost no matmul
# work remains after the final tile's completion sem fires.
PAIRS = (
    (0, 1024, (1, 1, 1, 1) + (2,) * 14),
    (1024, 512, (4,) * 8),
    (1536, 512, (4,) * 7 + (2, 1, 1)),
)
HOIST = 2                           # next-pair tiles prefetched before boundary
N_SUB = T_CORE // 128               # 16 output sub-tiles of 128 tokens
LO_SCALE = np.float32(2.0 ** 11)
LO_UNSCALE = 2.0 ** -11

_PROGRAM = None


def _split_multi_waits(nc):
    """walrus in this container rejects instructions carrying more sync waits
    than their ISA struct holds; normalize to a single wait by hoisting
    extras onto same-engine NOPs inserted immediately before the owner."""
    import bass_rust

    inserts = {}
    for f in nc.m.functions:
        for bb in f.blocks:
            for inst in bb.instructions:
                si = inst.sync_info
                if si is None or len(si.on_wait) <= 1:
                    continue
                conds = list(si.on_wait)
                si.on_wait = conds[:1]
                eng = nc.engines[inst.engine]
                new_insts = []
                for w in conds[1:]:
                    nop = eng.nop(hint="split_wait")
                    nop.ins.sync_info = bass_rust.SyncInfo(
                        on_wait=[w], on_update=[]
                    )
                    new_insts.append(nop.ins)
                inserts[inst.name] = new_insts
    if not inserts:
        return
    appended = {ni.name for nis in inserts.values() for ni in nis}
    for f in nc.m.functions:
        for bb in f.blocks:
            rebuilt = []
            changed = False
            for inst in bb.instructions:
                if inst.name in appended:
                    changed = True
                    continue
                if inst.name in inserts:
                    rebuilt.extend(inserts[inst.name])
                    changed = True
                rebuilt.append(inst)
            if changed:
                bb.instructions = rebuilt


def _build_program():
    import concourse.bass as bass
    import concourse.mybir as mybir
    import concourse.tile as tile

    f32 = mybir.dt.float32
    f16 = mybir.dt.float16
    u32 = mybir.dt.uint32
    E = NUM_EXPERTS

    nc = bass.Bass("TRN2", target_bir_lowering=False, debug=False)

    total_cols = N_CHUNK * 2 * T_CORE                     # 131072 f16 / part
    xdev = nc.dram_tensor("xdev", [128, total_cols], f16, kind="ExternalInput")
    phpl_d = nc.dram_tensor("phpl", [128, N_CHUNK * 2 * E], f16,
                            kind="ExternalInput")
    gate_neg = nc.dram_tensor("gate_neg", [E, 1], f32, kind="ExternalInput")
    w_out = nc.dram_tensor("w_out", [128, N_SUB * TOP_K], f32, kind="ExternalOutput")
    i_out = nc.dram_tensor("i_out", [128, N_SUB * TOP_K], u32, kind="ExternalOutput")

    ident_dram = nc.inline_tensor(np.eye(E, dtype=np.float32), name="ident64")

    with tile.TileContext(nc) as tc:
        with (
            tc.tile_pool(name="const", bufs=1) as const_pool,
            tc.tile_pool(name="xa", bufs=14) as x_pool,
            tc.tile_pool(name="acc", bufs=4, space="PSUM") as acc_pool,
            tc.tile_pool(name="bacc", bufs=3, space="PSUM") as b_pool,
            tc.tile_pool(name="tp", bufs=1, space="PSUM") as tp_pool,
            tc.tile_pool(name="lg", bufs=4) as lg_pool,
            tc.tile_pool(name="tk", bufs=4) as tk_pool,
            tc.tile_pool(name="outp", bufs=1) as out_pool,
        ):
            # --- constants.  Proto weights go as quarters interleaved with
            # the first x tiles on both rings, so weight delivery stays ~8
            # chunks ahead of the x stream without gating the first matmul.
            phpl_sb = const_pool.tile([128, N_CHUNK * 2 * E], f16)
            WQ = N_CHUNK // 4 * 2 * E                     # weight quarter cols
            nc.sync.dma_start(phpl_sb[:, 0:WQ], phpl_d[:, 0:WQ])
            gate_sb = const_pool.tile([E, 1], f32)
            nc.scalar.dma_start(gate_sb[:], gate_neg[:])
            ident_sb = const_pool.tile([E, E], f32)
            nc.scalar.dma_start(ident_sb[:], ident_dram[:])

            vals_sb = out_pool.tile([128, N_SUB * TOP_K], f32)
            idx_sb = out_pool.tile([128, N_SUB * TOP_K], u32)

            # PE warm-up: the HAM clock gate boots at 1.2GHz and needs
            # ~3.4us of sustained matmul activity to unthrottle.  Without
            # this, whether the kernel lands in the warm-PE equilibrium or a
            # ~12us-slower HAM-cold one depends on ramp timing luck.  These
            # matmuls depend only on the first weight quarter (lands ~10us,
            # while the first x tile is still in flight) and write a scratch
            # PSUM tile nobody reads.
            warm = tp_pool.tile([128, 4 * E], f32, name="warm", tag="tkp")
            for w in range(24):
                nc.tensor.matmul(
                    warm[:, :], phpl_sb[:, 0:128], phpl_sb[:, 0:4 * E],
                    start=(w == 0), stop=(w == 23),
                )

            # x tile column offsets in xdev (pair-major layout)
            tile_off = {}
            o = 0
            for p, (toff, pw, cpts) in enumerate(PAIRS):
                for j, cpt in enumerate(cpts):
                    tile_off[(p, j)] = o
                    o += cpt * 2 * pw

            prefetched = {}

            def emit_xdma(p, j):
                # all x triggers ride the sync ring: the scalar ENGINE runs
                # the epilogue ACTs (a trigger queued behind them would stall
                # the stream at pair boundaries), and the scalar ring's first
                # packets start ~3.4us late, which perturbs the ramp enough
                # to trip the HAM clock gate
                pw, cpt = PAIRS[p][1], PAIRS[p][2][j]
                tile_cols = cpt * 2 * pw
                off = tile_off[(p, j)]
                x_t = x_pool.tile([128, 4096], f16, name="x_t", tag="xt")
                if cpt > 1:
                    # two chunk-aligned DMAs per tile: completion sems then
                    # arrive in 512KB quanta, so PE wait episodes stay under
                    # the ~1.4us HAM re-throttle threshold even with HBM
                    # receipt-latency jitter
                    half = tile_cols // 2
                    nc.sync.dma_start(x_t[:, 0:half], xdev[:, off:off + half])
                    nc.sync.dma_start(x_t[:, half:tile_cols],
                                      xdev[:, off + half:off + tile_cols])
                else:
                    nc.sync.dma_start(x_t[:, 0:tile_cols],
                                      xdev[:, off:off + tile_cols])
                return x_t

            for p, (toff, pw, cpts) in enumerate(PAIRS):
                W = pw // 2
                tpp = len(cpts)
                accs = [
                    acc_pool.tile([128, W], f32, name=f"acc_p{p}{h}", tag="acc")
                    for h in range(2)
                ]
                baccs = [
                    b_pool.tile([128, W], f32, name=f"bacc_p{p}{h}", tag="bacc")
                    for h in range(2)
                ]
                c0 = 0
                for j in range(tpp):
                    cpt = cpts[j]
                    # hoist the next pair's first tiles ahead of this pair's
                    # last tiles so the PE sees no delivery gap at the
                    # boundary (a >1us PE idle trips the HAM clock gate)
                    if p + 1 < len(PAIRS) and j >= tpp - HOIST:
                        hj = j - (tpp - HOIST)
                        prefetched[(p + 1, hj)] = emit_xdma(p + 1, hj)
                    if (p, j) in prefetched:
                        x_t = prefetched.pop((p, j))
                    else:
                        x_t = emit_xdma(p, j)
                    if p == 0 and j in (0, 1, 2):
                        q = j + 1
                        nc.sync.dma_start(
                            phpl_sb[:, q * WQ:(q + 1) * WQ],
                            phpl_d[:, q * WQ:(q + 1) * WQ])
                    for k in range(cpt):
                        c = c0 + k
                        pc = slice(c * 2 * E, (c + 1) * 2 * E)
                        first, last = (c == 0), (c == N_CHUNK - 1)
                        for h in range(2):
                            rh = x_t[:, (2 * k) * pw + h * W:
                                     (2 * k) * pw + (h + 1) * W]
                            rl = x_t[:, (2 * k + 1) * pw + h * W:
                                     (2 * k + 1) * pw + (h + 1) * W]
                            nc.tensor.matmul(
                                accs[h][:, :], phpl_sb[:, pc], rh,
                                start=first, stop=last,
                            )
                            nc.tensor.matmul(
                                baccs[h][:, :], phpl_sb[:, pc], rl,
                                start=first, stop=last,
                            )
                    c0 += cpt
                # epilogue per group in <=256-col slices (128-aligned)
                for h in range(2):
                    acc, bacc = accs[h], baccs[h]
                    nsub = W // 128
                    tkp = tp_pool.tile([128, nsub * E], f32,
                                       name=f"tk_psum_p{p}{h}", tag="tkp")
                    s0 = (toff + h * W) // 128
                    for e0 in range(0, W, 256):
                        ew = min(256, W - e0)
                        hs = slice(e0, e0 + ew)
                        hi_sb = lg_pool.tile([E, 256], f32, name="hi_sb")
                        nc.scalar.activation(
                            hi_sb[0:E, 0:ew], acc[0:E, hs],
                            bass.mybir.ActivationFunctionType.Copy,
                            scale=1.0 / 64.0,
                        )
                        u_sb = lg_pool.tile([E, 256], f32, name="u_sb")
                        nc.vector.scalar_tensor_tensor(
                            u_sb[0:E, 0:ew], acc[E:2 * E, hs],
                            LO_UNSCALE / 64.0, hi_sb[0:E, 0:ew],
                            bass.mybir.AluOpType.mult, bass.mybir.AluOpType.add,
                        )
                        comb = lg_pool.tile([E, 256], f32, name="comb")
                        nc.vector.scalar_tensor_tensor(
                            comb[0:E, 0:ew], bacc[0:E, hs],
                            LO_UNSCALE / 64.0, u_sb[0:E, 0:ew],
                            bass.mybir.AluOpType.mult, bass.mybir.AluOpType.add,
                        )
                        logits = lg_pool.tile([E, 256], f32, name="logits")
                        nc.scalar.activation(
                            logits[0:E, 0:ew], comb[0:E, 0:ew],
                            bass.mybir.ActivationFunctionType.Relu,
                            bias=gate_sb[:], scale=1.0,
                        )
                        for q in range(ew // 128):
                            sl = (e0 + q * 128) // 128
                            nc.tensor.transpose(
                                tkp[:, sl * E:(sl + 1) * E],
                                logits[:, q * 128:(q + 1) * 128],
                                ident_sb[:],
                            )
                        # copy transposes out of PSUM promptly (the PE's next
                        # transpose into tkp must not wait on DVE max ops —
                        # it would stall the in-order PE queue)
                        tk_sb = tk_pool.tile([128, 2 * E], f32, name="tk_sb")
                        nc.vector.tensor_copy(
                            tk_sb[:, 0:(ew // 128) * E],
                            tkp[:, (e0 // 128) * E:((e0 + ew) // 128) * E])
                        for q in range(ew // 128):
                            s = s0 + (e0 + q * 128) // 128
                            nc.vector.max(
                                vals_sb[:, s * TOP_K:(s + 1) * TOP_K],
                                tk_sb[:, q * E:(q + 1) * E],
                            )
                            nc.vector.max_index(
                                idx_sb[:, s * TOP_K:(s + 1) * TOP_K],
                                vals_sb[:, s * TOP_K:(s + 1) * TOP_K],
                                tk_sb[:, q * E:(q + 1) * E],
                            )
                    os_ = slice(s0 * TOP_K, (s0 + W // 128) * TOP_K)
                    if p < len(PAIRS) - 1:
                        # mid-stream flushes ride the (otherwise idle) gpsimd
                        # SWDGE ring so they queue behind neither the x
                        # stream nor the ACTs
                        nc.gpsimd.dma_start(w_out[:, os_], vals_sb[:, os_])
                        nc.gpsimd.dma_start(i_out[:, os_], idx_sb[:, os_])
                    else:
                        # final flushes sit on the kernel's critical tail:
                        # flush per group as soon as its maxes land, with
                        # w/i on different rings (both idle by now) so the
                        # triggers and their ~2us HBM completion receipts
                        # overlap instead of serializing
                        nc.scalar.dma_start(w_out[:, os_], vals_sb[:, os_])
                        nc.sync.dma_start(i_out[:, os_], idx_sb[:, os_])

    _split_multi_waits(nc)
    return nc


def _get_program():
    global _PROGRAM
    if _PROGRAM is None:
        _PROGRAM = _build_program()
    return _PROGRAM


def _make_in_maps(x, proto_k, gate):
    xf = np.ascontiguousarray(x, dtype=np.float32).reshape(TOKENS, HIDDEN)
    proto = np.asarray(proto_k, dtype=np.float32)
    ph = proto.astype(np.float16)
    pl = ((proto - ph.astype(np.float32)) * LO_SCALE).astype(np.float16)
    # phpl[p, c*128+m]: W = [ph; pl] rows = 128 packed expert cols
    Wm = np.concatenate([ph, pl], axis=0)                 # [128, 4096]
    phpl = np.ascontiguousarray(
        Wm.T.reshape(N_CHUNK, 128, 2 * NUM_EXPERTS).transpose(1, 0, 2)
        .reshape(128, N_CHUNK * 2 * NUM_EXPERTS)
    )
    gate_neg = np.ascontiguousarray(
        -np.asarray(gate, dtype=np.float32).reshape(NUM_EXPERTS, 1)
    )
    in_maps = []
    for cid in range(N_CORES):
        xs = xf[cid * T_CORE:(cid + 1) * T_CORE]          # [2048, 4096]
        xh = xs.astype(np.float16)
        xl = ((xs - xh.astype(np.float32)) * LO_SCALE).astype(np.float16)
        # A[c, p, s, t] = (xh if s==0 else xl)[t, c*128+p]
        A = np.empty((N_CHUNK, 128, 2, T_CORE), np.float16)
        A[:, :, 0, :] = xh.T.reshape(N_CHUNK, 128, T_CORE)
        A[:, :, 1, :] = xl.T.reshape(N_CHUNK, 128, T_CORE)
        # delivery: pair-major, tiles of cpts[j] chunks: cols [p][j][k][s][t]
        parts = []
        for toff, pw, cpts in PAIRS:
            c0 = 0
            for cpt in cpts:
                blk = A[c0:c0 + cpt, :, :, toff:toff + pw]
                parts.append(blk.transpose(1, 0, 2, 3).reshape(128, -1))
                c0 += cpt
        xd = np.ascontiguousarray(np.concatenate(parts, axis=1))
        in_maps.append({"xdev": xd, "phpl": phpl, "gate_neg": gate_neg})
    return in_maps


def _gather(results):
    w = np.empty((TOKENS, TOP_K), np.float32)
    idx = np.empty((TOKENS, TOP_K), np.int32)
    for c in range(N_CORES):
        wo = results[c]["w_out"]                          # [128, 16*8]
        io = results[c]["i_out"].view(np.int32)
        w[c * T_CORE:(c + 1) * T_CORE] = (
            wo.reshape(128, N_SUB, TOP_K).transpose(1, 0, 2).reshape(T_CORE, TOP_K)
        )
        idx[c * T_CORE:(c + 1) * T_CORE] = (
            io.reshape(128, N_SUB, TOP_K).transpose(1, 0, 2).reshape(T_CORE, TOP_K)
        )
    return w.reshape(4, 4096, TOP_K), idx.reshape(4, 4096, TOP_K)


def run_sharded(in_maps, trace=False, trace_cores=None):
    from concourse.bass_utils import run_bass_kernel_spmd

    nc = _get_program()
    return run_bass_kernel_spmd(
        nc,
        in_maps,
        core_ids=list(range(N_CORES)),
        trace=trace,
        trace_cores=trace_cores,
    )


def kernel(x, proto_k, gate):
    in_maps = _make_in_maps(x, proto_k, gate)
    res = run_sharded(in_maps, trace=False)
    return _gather(res.results)
